# revision 64
# baseline (speedup 1.0000x reference)
"""FFF (fast feedforward / MoE tree-routing) Trainium2 kernel.

Strategy (8 NeuronCores, SPMD, two launches):
  Launch 1 — routing, data-parallel over batch: each core routes 1024 samples
    through the depth-11 plane tree. Levels 0..7 are evaluated densely
    (fp32 matmuls of x against 255 node planes; per-sample select via a single
    fused (iota==cur)*score scalar_tensor_tensor with accum per level).
    Levels 8..10 gather each sample's [w|b] node row with bulk SWDGE
    dma_gathers; the gather index vector is relayouted via a tiny DRAM
    round trip (1 write + 1 wrapped read) and replicated across the 8
    gpsimd core groups with a constant [16,128] PE matmul. Per-sample dots
    are single fused DVE scalar_tensor_tensor+accum ops. Four independent
    quarter-pipelines overlap gather DMA with other quarters' compute.
  Host — slot assignment: samples grouped by leaf expert; leaves sharded
    expert-parallel 256/core; experts are LPT-packed into 32 groups of 8 to
    balance per-group sample counts; capacity spg chosen from the actual
    distribution. x rows gathered+transposed on the host (bf16).
  Launch 2 — expert MLP, expert-parallel, bf16: per 8-expert group one fused
    [768x128] @ [768xspg] bf16 matmul chain computes all 8 experts' h lanes,
    relu+bias on ACT, lane-mask on DVE, then h.T @ W2stack (bf16) produces
    output rows; PSUM drains split ACT/DVE; bf16 stores. Weights stream
    through SBUF once per core (12.6 MB bf16).
  Host — scatter output rows back to sample order (fp32).
"""

import contextlib
import numpy as np
import ml_dtypes

import concourse.bacc as bacc
import concourse.mybir as mybir
import concourse.tile as tile
from concourse.bass import ts
from concourse.mybir import AluOpType
from concourse.bass_utils import run_bass_kernel_spmd

# problem shapes (hardcoded per contract)
DEPTH = 11
IN_W = 768
LEAF_W = 16
OUT_W = 768
N_NODES = 2047
N_LEAVES = 2048
BATCH = 8192
N_CORES = 8

# routing kernel layout
B_CORE = BATCH // N_CORES            # 1024
EXT = 832                            # gather row [w(768) | b | pad] (3328B, %256)
DOT = IN_W + 1                       # useful columns of a gathered row
DENSE_LEVELS = 8                     # levels 0..7 dense (255 nodes)
N_DENSE = 2 ** DENSE_LEVELS - 1      # 255
KC = IN_W // 128                     # 6 contraction k-tiles
NQ = 8                               # routing quarter pipelines
CQ = 8 // NQ                         # c-tiles per quarter (2)
QN = B_CORE // NQ                    # samples per quarter (256)

# mlp kernel layout
LEAVES_PER_CORE = N_LEAVES // N_CORES           # 256
EXPERTS_PER_GROUP = 8
GROUPS = LEAVES_PER_CORE // EXPERTS_PER_GROUP   # 32
SLOTS_PER_GROUP = 64                            # default capacity (exact spg
                                                # picked from the routing result)

F32 = mybir.dt.float32
I32 = mybir.dt.int32
I16 = mybir.dt.int16
BF16 = mybir.dt.bfloat16
BF = ml_dtypes.bfloat16

LAST_SPG = SLOTS_PER_GROUP   # capacity used by the most recent kernel() call


# ---------------------------------------------------------------- launch 1
def _build_routing_nc():
    nc = bacc.Bacc("TRN2", target_bir_lowering=False, debug=False,
                   num_devices=N_CORES)
    xT = nc.dram_tensor("xT", [IN_W, B_CORE], F32, kind="ExternalInput").ap()
    xe = nc.dram_tensor("xe", [B_CORE, DOT], F32, kind="ExternalInput").ap()
    wd = nc.dram_tensor("wd", [IN_W, 256], F32, kind="ExternalInput").ap()
    brep = nc.dram_tensor("brep", [128, 256], F32, kind="ExternalInput").ap()
    RT2 = nc.dram_tensor("RT2", [128, 128], F32, kind="ExternalInput").ap()
    M8 = nc.dram_tensor("M8", [128, CQ * 8], F32, kind="ExternalInput").ap()
    nwe = nc.dram_tensor("nwe", [N_LEAVES, EXT], F32, kind="ExternalInput").ap()
    leaf = nc.dram_tensor("leaf", [B_CORE], I32, kind="ExternalOutput").ap()

    xT_r = xT.rearrange("(k p) s -> p k s", p=128)
    xe_r = xe.rearrange("(c p) d -> p c d", p=128)

    with tile.TileContext(nc) as tc, contextlib.ExitStack() as ctx:
        pool = ctx.enter_context(tc.tile_pool(name="sbuf", bufs=1))
        wpool = ctx.enter_context(tc.tile_pool(name="work", bufs=2))
        cpool = ctx.enter_context(tc.tile_pool(name="cwork", bufs=3))
        psum = ctx.enter_context(tc.tile_pool(name="psum", bufs=1, space="PSUM"))
        psr = ctx.enter_context(tc.tile_pool(name="psr", bufs=2, space="PSUM"))

        # ---- PE warm-up: garbage matmuls ramp the tensor engine to full
        # p-state while the input DMAs stream, so the dense chains that gate
        # the whole kernel run at 1x cycle time from their first instruction.
        psjp = ctx.enter_context(tc.tile_pool(name="psj", bufs=1, space="PSUM"))
        junk = pool.tile([128, 256], F32, name="junk")
        nc.vector.memset(junk[:], 0)
        psj = psjp.tile([128, 128], F32, space="PSUM", name="psjunk")
        for i in range(9):
            nc.tensor.matmul(psj[:], lhsT=junk[:, :128], rhs=junk[:, 128:],
                             start=(i == 0), stop=(i == 8),
                             skip_group_check=True)

        # ---- loads (SP queue, no waits) in consumption order
        wd_sb = pool.tile([128, KC, 256], F32)
        nc.sync.dma_start(out=wd_sb[:], in_=wd.rearrange("(k p) n -> p k n", p=128))
        brep_sb = pool.tile([128, 256], F32)
        nc.sync.dma_start(out=brep_sb[:], in_=brep[:])
        RT2_sb = pool.tile([128, 128], F32)
        nc.sync.dma_start(out=RT2_sb[:], in_=RT2[:])
        M8_sb = pool.tile([128, CQ, 8], F32)
        nc.sync.dma_start(out=M8_sb[:], in_=M8.rearrange("p (c g) -> p c g", g=8))
        xT_q = []
        for q in range(NQ):
            t = pool.tile([128, KC, QN], F32, name=f"xT{q}")
            nc.sync.dma_start(out=t[:], in_=xT_r[:, :, ts(q, QN)])
            xT_q.append(t)
        xe_q = []
        for q in range(NQ):
            t = pool.tile([128, CQ, DOT], F32, name=f"xe{q}")
            nc.sync.dma_start(out=t[:], in_=xe_r[:, ts(q, CQ), :])
            xe_q.append(t)

        # iota of global node index (values = column j)
        iota_i = pool.tile([128, 256], I32)
        nc.gpsimd.iota(iota_i[:], pattern=[[1, 256]], base=0, channel_multiplier=0)
        iota_f = pool.tile([128, 256], F32)
        nc.vector.tensor_copy(out=iota_f[:], in_=iota_i[:])

        # ---- dense matmuls, interleaved with per-quarter trip matmuls so a
        # quarter's level-8 gather issues as soon as its two c-tiles are done
        ps_c = {}

        # split each c-tile's node columns at level boundaries so selects for
        # levels 1..5 start as soon as the first 63-col chain stops
        CH = ((0, 63), (63, 127), (127, 256))

        def dense_mm(c):
            ps = psum.tile([128, 256], F32, space="PSUM", tag=f"ps{c % 3}",
                           name=f"ps{c}")
            for lo, hi in CH:
                for k in range(KC):
                    nc.tensor.matmul(ps[:, lo:hi],
                                     lhsT=xT_q[c // CQ][:, k, ts(c % CQ, 128)],
                                     rhs=wd_sb[:, k, lo:hi], start=(k == 0),
                                     stop=(k == KC - 1))
            ps_c[c] = ps

        # ---- dense selects per quarter (DVE); cur lives in a shared
        # [128, 8] tile (per-quarter column slices) so one DMA per level can
        # export all quarters' indices.
        s_q, sel_q, ch_q = {}, {}, {}
        scr_sel, dot_scr = {}, {}
        cur_l = {0: cpool.tile([128, 8], F32, tag="cur", name="curl0")}
        for q in range(NQ):
            s_q[q] = pool.tile([128, CQ, 256], F32, name=f"s{q}")
            sel_q[q] = pool.tile([128, CQ], F32, name=f"sel{q}")
            ch_q[q] = pool.tile([128, CQ], F32, name=f"ch{q}")
            scr_sel[q] = pool.tile([128, 256], F32, name=f"scrsel{q}")
            dot_scr[q] = pool.tile([128, DOT], F32, name=f"dotscr{q}")
        for lvl in range(1, DENSE_LEVELS):
            cur_l[lvl] = cpool.tile([128, 8], F32, tag="cur", name=f"curl{lvl}")

        def sel_engine(q):
            # odd eighths run their select chains on the (otherwise idle)
            # gpsimd engine so the two sets of chains advance in parallel
            return nc.vector

        def dense_selects(q):
            eng = sel_engine(q)
            for lo, hi in CH:
                for c in range(CQ):
                    # s = psum + bias (per-node row, replicated on partitions)
                    eng.scalar_tensor_tensor(
                        out=s_q[q][:, c, lo:hi], in0=ps_c[CQ * q + c][:, lo:hi],
                        scalar=0.0, in1=brep_sb[:, lo:hi], op0=AluOpType.add,
                        op1=AluOpType.add)
                if lo == 0:
                    # level 0: cur = (s[:,:,0] >= 0) + 1
                    eng.tensor_scalar(
                        out=cur_l[0][:, ts(q, CQ)], in0=s_q[q][:, :, 0],
                        scalar1=0.0, scalar2=1.0, op0=AluOpType.is_ge,
                        op1=AluOpType.add)
                    for lvl in range(1, 6):
                        level_select(q, lvl)
                elif lo == 63:
                    level_select(q, 6)
                else:
                    level_select(q, 7)

        def level_select(q, lvl):
            eng = sel_engine(q)
            if True:
                n = 2 ** lvl
                off = n - 1
                for c in range(CQ):
                    # fused select: sum_j (iota_j == cur) * s_j
                    eng.scalar_tensor_tensor(
                        out=scr_sel[q][:, :n], in0=iota_f[:, off:off + n],
                        scalar=cur_l[lvl - 1][:, CQ * q + c:CQ * q + c + 1],
                        in1=s_q[q][:, c, off:off + n],
                        op0=AluOpType.is_equal, op1=AluOpType.mult,
                        accum_out=sel_q[q][:, c:c + 1])
                eng.tensor_scalar(
                    out=ch_q[q][:], in0=sel_q[q][:], scalar1=0.0, scalar2=1.0,
                    op0=AluOpType.is_ge, op1=AluOpType.add)
                eng.scalar_tensor_tensor(
                    out=cur_l[lvl][:, ts(q, CQ)],
                    in0=cur_l[lvl - 1][:, ts(q, CQ)], scalar=2.0,
                    in1=ch_q[q][:], op0=AluOpType.mult, op1=AluOpType.add)

        # ---- deep levels 8..10: four quarter pipelines, all on-chip.
        # The wrapped [16, 16] gather-index layout is produced by a matmul
        # fold (rhs = cur broadcast * M8 lane mask, F = R'^T @ rhs puts
        # cur[16g+l, c] at [l, 8c+g]), then a second matmul replicates it to
        # the 8 gpsimd core groups. No DMA round trips.
        def q_trip(lvl, q, prev):
            rhs = wpool.tile([128, CQ, 8], F32, tag=f"rhs{q}",
                             name=f"rhs{q}l{lvl}")
            nc.vector.tensor_tensor(
                out=rhs[:], in0=prev[:, ts(q, CQ), None].to_broadcast(
                    [128, CQ, 8]),
                in1=M8_sb[:], op=AluOpType.mult)
            # single fold+replicate matmul: out[m, 8c+g] = cur[16g + m%16, c]
            prep = psr.tile([128, QN // 16], F32, space="PSUM", tag="pr",
                            name=f"pr{q}l{lvl}")
            nc.tensor.matmul(prep[:], lhsT=RT2_sb[:],
                             rhs=rhs.rearrange("p c g -> p (c g)"),
                             start=True, stop=True)
            idx16 = wpool.tile([128, QN // 16], I16, tag=f"ix{q}",
                               name=f"ix{q}l{lvl}")
            nc.vector.tensor_copy(out=idx16[:], in_=prep[:])
            gath = wpool.tile([128, CQ, EXT], F32, tag=f"g{q}",
                              name=f"g{q}l{lvl}")
            nc.gpsimd.dma_gather(
                out_ap=gath[:], in_ap=nwe[:], idxs_ap=idx16[:],
                num_idxs=QN, num_idxs_reg=QN, elem_size=EXT)
            return gath

        def q_dots(lvl, q, prev, nxt, gath):
            for c in range(CQ):
                # fused dot: sum_i xe_i * w_i  (incl. ones-col * bias)
                nc.vector.scalar_tensor_tensor(
                    out=dot_scr[q][:], in0=xe_q[q][:, c, :],
                    scalar=1.0, in1=gath[:, c, :DOT],
                    op0=AluOpType.mult, op1=AluOpType.mult,
                    accum_out=sel_q[q][:, c:c + 1])
            nc.vector.tensor_scalar(
                out=ch_q[q][:], in0=sel_q[q][:], scalar1=0.0, scalar2=1.0,
                op0=AluOpType.is_ge, op1=AluOpType.add)
            nc.vector.scalar_tensor_tensor(
                out=nxt[:, ts(q, CQ)], in0=prev[:, ts(q, CQ)], scalar=2.0,
                in1=ch_q[q][:], op0=AluOpType.mult, op1=AluOpType.add)

        for lvl in range(DENSE_LEVELS, DEPTH):
            cur_l[lvl] = cpool.tile([128, 8], F32, tag="cur", name=f"curl{lvl}")

        gaths = {}
        cur7 = cur_l[DENSE_LEVELS - 1]
        for c in range(CQ):
            dense_mm(c)
        for q in range(NQ):
            if q + 1 < NQ:
                for c in range(CQ * (q + 1), CQ * (q + 2)):
                    dense_mm(c)
            dense_selects(q)
            gaths[q] = q_trip(DENSE_LEVELS, q, cur7)
        for lvl in range(DENSE_LEVELS, DEPTH):
            for q in range(NQ):
                q_dots(lvl, q, cur_l[lvl - 1], cur_l[lvl], gaths[q])
                if lvl + 1 < DEPTH:
                    gaths[q] = q_trip(lvl + 1, q, cur_l[lvl])

        # ---- leaves = cur - N_NODES
        leaf_i = pool.tile([128, 8], I32)
        nc.vector.tensor_scalar(
            out=leaf_i[:], in0=cur_l[DEPTH - 1][:],
            scalar1=float(N_NODES), scalar2=None, op0=AluOpType.subtract)
        nc.sync.dma_start(out=leaf.rearrange("(c p) -> p c", p=128), in_=leaf_i[:])

    nc.compile()
    return nc


def _host_prep_routing(x, node_weights, node_biases):
    wd = np.zeros((IN_W, 256), np.float32)
    wd[:, :N_DENSE] = node_weights[:N_DENSE].T
    brep = np.zeros((128, 256), np.float32)
    brep[:, :N_DENSE] = node_biases[None, :N_DENSE]
    RT2 = (np.arange(128)[:, None] % 16 ==
           np.arange(128)[None, :] % 16).astype(np.float32)
    M8 = np.zeros((128, CQ * 8), np.float32)
    for c in range(CQ):
        M8[np.arange(128), 8 * c + np.arange(128) // 16] = 1.0
    nwe = np.zeros((N_LEAVES, EXT), np.float32)
    nwe[:N_NODES, :IN_W] = node_weights
    nwe[:N_NODES, IN_W] = node_biases

    in_maps = []
    for c in range(N_CORES):
        xs = x[c * B_CORE:(c + 1) * B_CORE]
        xT = np.ascontiguousarray(xs.T)
        xe = np.ones((B_CORE, DOT), np.float32)
        xe[:, :IN_W] = xs
        in_maps.append({"xT": xT, "xe": xe, "wd": wd, "brep": brep,
                        "RT2": RT2, "M8": M8, "nwe": nwe})
    return in_maps


# ---------------------------------------------------------------- launch 2
def _build_mlp_nc(spg=SLOTS_PER_GROUP):
    SLOTS = GROUPS * spg
    NH = OUT_W // 2
    nc = bacc.Bacc("TRN2", target_bir_lowering=False, debug=False,
                   num_devices=N_CORES)
    xgT = nc.dram_tensor("xgT", [IN_W, SLOTS], BF16, kind="ExternalInput").ap()
    wslab = nc.dram_tensor("wslab", [GROUPS, 128, KC * 128 + OUT_W], BF16,
                           kind="ExternalInput").ap()
    b1bc = nc.dram_tensor("b1bc", [128, GROUPS], F32, kind="ExternalInput").ap()
    maskt = nc.dram_tensor("maskt", [128, SLOTS], BF16, kind="ExternalInput").ap()
    out = nc.dram_tensor("o", [SLOTS, OUT_W], BF16, kind="ExternalOutput").ap()

    xgT_r = xgT.rearrange("(k p) s -> p k s", p=128)
    wslab_r = wslab.rearrange("g p m -> p g m")
    NW = 4                      # weight groups per slab DMA
    NSLAB = GROUPS // NW        # 8 slab DMAs
    pair_stores = (spg == 64)   # 2-group packed stores need partition offset 64

    with tile.TileContext(nc) as tc, contextlib.ExitStack() as ctx:
        pool = ctx.enter_context(tc.tile_pool(name="sbuf", bufs=1))
        wpool = ctx.enter_context(tc.tile_pool(name="w", bufs=NSLAB))
        hpool = ctx.enter_context(tc.tile_pool(name="h", bufs=3))
        opool = ctx.enter_context(tc.tile_pool(name="o", bufs=6))
        ps1 = ctx.enter_context(tc.tile_pool(name="ps1", bufs=3, space="PSUM"))
        ps2 = ctx.enter_context(tc.tile_pool(name="ps2", bufs=2, space="PSUM"))

        # loads: interleave xt chunks and weight slabs in consumption order
        xt_sb = []
        mask_sb = pool.tile([128, SLOTS], BF16)
        b1_sb = pool.tile([128, GROUPS], F32)
        w_sb = []
        xt0 = pool.tile([128, KC, NW * spg], BF16, name="xt0")
        nc.sync.dma_start(out=xt0[:], in_=xgT_r[:, :, :NW * spg])
        xt_sb.append(xt0)
        nc.sync.dma_start(out=mask_sb[:], in_=maskt[:])
        nc.sync.dma_start(out=b1_sb[:], in_=b1bc[:])
        for j in range(NSLAB):
            w4 = wpool.tile([128, NW, KC * 128 + OUT_W], BF16, tag="w",
                            name=f"w4_{j}")
            if j == NSLAB - 1:
                for r in range(NW):
                    nc.sync.dma_start(out=w4[:, r, :],
                                      in_=wslab_r[:, j * NW + r, :])
            else:
                nc.sync.dma_start(out=w4[:], in_=wslab_r[:, ts(j, NW), :])
            w_sb.append(w4)
            if j + 1 < NSLAB:
                lo = (j + 1) * NW * spg
                xtj = pool.tile([128, KC, NW * spg], BF16, name=f"xt{j + 1}")
                nc.sync.dma_start(out=xtj[:], in_=xgT_r[:, :, lo:lo + NW * spg])
                xt_sb.append(xtj)

        # pipelined group loop: w1(t) | w2(t-1) | relu+mask(t) | drain(t-2)
        p1_t, hf_t, p2_t, o_t = {}, {}, {}, {}

        def w1mm(g):
            w1 = w_sb[g // NW][:, g % NW, :KC * 128]
            p1 = ps1.tile([128, spg], F32, space="PSUM", tag="p1",
                          name=f"p1_{g}")
            for k in range(KC):
                nc.tensor.matmul(p1[:], lhsT=w1[:, ts(k, 128)],
                                 rhs=xt_sb[g // NW][:, k, ts(g % NW, spg)],
                                 start=(k == 0), stop=(k == KC - 1))
            p1_t[g] = p1

        def hcompute(g):
            h1 = hpool.tile([128, spg], BF16, tag="h1", name=f"h1_{g}")
            nc.scalar.activation(out=h1[:], in_=p1_t[g][:],
                                 func=mybir.ActivationFunctionType.Relu,
                                 bias=b1_sb[:, g:g + 1])
            hf = hpool.tile([128, spg], BF16, tag="hf", name=f"hf_{g}")
            nc.gpsimd.tensor_tensor(out=hf[:], in0=h1[:],
                                    in1=mask_sb[:, ts(g, spg)],
                                    op=AluOpType.mult)
            hf_t[g] = hf

        def w2mm(g):
            w2 = w_sb[g // NW][:, g % NW, KC * 128:]
            p2a = ps2.tile([spg, NH], F32, space="PSUM", tag="p2a",
                           name=f"p2a_{g}")
            p2b = ps2.tile([spg, NH], F32, space="PSUM", tag="p2b",
                           name=f"p2b_{g}")
            nc.tensor.matmul(p2a[:], lhsT=hf_t[g][:], rhs=w2[:, :NH],
                             start=True, stop=True)
            nc.tensor.matmul(p2b[:], lhsT=hf_t[g][:], rhs=w2[:, NH:],
                             start=True, stop=True)
            p2_t[g] = (p2a, p2b)

        def drain(g):
            p2a, p2b = p2_t.pop(g)
            if pair_stores:
                if g % 2 == 0:
                    o_t[g] = opool.tile([128, OUT_W], BF16, tag="o",
                                        name=f"o_{g}")
                o_sb = o_t[g - g % 2]
                off = (g % 2) * spg
                nc.scalar.copy(out=o_sb[off:off + spg, :NH], in_=p2a[:])
                nc.vector.tensor_copy(out=o_sb[off:off + spg, NH:], in_=p2b[:])
                if g % 2 == 1:
                    nc.sync.dma_start(out=out[ts(g // 2, 2 * spg), :],
                                      in_=o_sb[:])
            else:
                o_sb = opool.tile([spg, OUT_W], BF16, tag="o", name=f"o_{g}")
                nc.scalar.copy(out=o_sb[:, :NH], in_=p2a[:])
                nc.vector.tensor_copy(out=o_sb[:, NH:], in_=p2b[:])
                nc.sync.dma_start(out=out[ts(g, spg), :], in_=o_sb[:])

        for t in range(GROUPS + 2):
            if t < GROUPS:
                w1mm(t)
            if 1 <= t <= GROUPS:
                w2mm(t - 1)
            if t < GROUPS:
                hcompute(t)
            if t >= 2:
                drain(t - 2)

    nc.compile()
    return nc


def _host_prep_mlp(leaves, x, w1s, b1s, w2s, spg=SLOTS_PER_GROUP):
    SLOTS = GROUPS * spg
    in_maps, slot_maps = [], []
    order = np.argsort(leaves, kind="stable")
    sorted_leaves = leaves[order]
    for c in range(N_CORES):
        lo, hi = LEAVES_PER_CORE * c, LEAVES_PER_CORE * (c + 1)
        beg, end = np.searchsorted(sorted_leaves, [lo, hi])
        samples = order[beg:end]
        l_loc = leaves[samples] - lo

        # LPT pack experts into groups of 8 balancing sample counts
        loads = np.bincount(l_loc, minlength=LEAVES_PER_CORE)
        perm = _lpt_groups(loads)
        # position of each expert: group + lane-slot within group
        e_group = np.empty(LEAVES_PER_CORE, np.int64)
        e_pos = np.empty(LEAVES_PER_CORE, np.int64)
        for g in range(GROUPS):
            for p_, e in enumerate(perm[g * EXPERTS_PER_GROUP:
                                        (g + 1) * EXPERTS_PER_GROUP]):
                e_group[e] = g
                e_pos[e] = p_

        g_all = e_group[l_loc]
        p_all = e_pos[l_loc]
        slot = np.empty(len(samples), np.int64)
        fill = np.zeros(GROUPS, np.int64)
        for i, g in enumerate(g_all):
            slot[i] = spg * g + fill[g]
            fill[g] += 1
        assert not len(fill) or fill.max() <= spg

        slot_sample = np.full(SLOTS, -1, np.int64)
        slot_sample[slot] = samples
        mask = np.zeros((128, SLOTS), BF)
        lane_rows = (16 * p_all[None, :] + np.arange(16)[:, None])
        mask[lane_rows, slot[None, :]] = 1.0

        xg = np.zeros((SLOTS, IN_W), np.float32)
        xg[slot] = x[samples]
        xgT = np.ascontiguousarray(xg.T).astype(BF)

        eids = lo + perm                       # global leaf id per lane-slot
        w1sel = w1s[eids]                      # [256, 768, 16]
        w1f = (w1sel.reshape(GROUPS, 8, IN_W, LEAF_W)
               .transpose(0, 2, 1, 3)
               .reshape(GROUPS, IN_W, 128)
               .reshape(GROUPS, KC, 128, 128)
               .transpose(0, 2, 1, 3)
               .reshape(GROUPS, 128, KC * 128))
        w2f = w2s[eids].reshape(GROUPS, 128, OUT_W)
        wslab = np.ascontiguousarray(
            np.concatenate([w1f, w2f], axis=2)).astype(BF)
        b1bc = np.ascontiguousarray(
            b1s[eids].reshape(GROUPS, 128).T).astype(np.float32)

        in_maps.append({"xgT": xgT, "wslab": wslab, "b1bc": b1bc,
                        "maskt": mask})
        slot_maps.append(slot_sample)
    return in_maps, slot_maps


def _lpt_groups(loads):
    """Pack 256 experts into 32 groups of exactly 8, balancing total load
    (greedy LPT with cardinality cap). Returns expert permutation, 8 per
    group."""
    order = np.argsort(-loads, kind="stable")
    gload = np.zeros(GROUPS, np.int64)
    gcnt = np.zeros(GROUPS, np.int64)
    members = [[] for _ in range(GROUPS)]
    for e in order:
        gl = np.where(gcnt < EXPERTS_PER_GROUP, gload, np.iinfo(np.int64).max)
        g = int(np.argmin(gl))
        members[g].append(int(e))
        gload[g] += loads[e]
        gcnt[g] += 1
    return np.array([e for g in range(GROUPS) for e in members[g]],
                    dtype=np.int64)


# ---------------------------------------------------------------- entry
def kernel(x, node_weights, node_biases, w1s, b1s, w2s):
    x = np.ascontiguousarray(np.asarray(x, np.float32))
    node_weights = np.ascontiguousarray(np.asarray(node_weights, np.float32))
    node_biases = np.ascontiguousarray(np.asarray(node_biases, np.float32))
    w1s = np.asarray(w1s, np.float32)
    b1s = np.asarray(b1s, np.float32)
    w2s = np.asarray(w2s, np.float32)

    # launch 1: routing (retry once if a transient device glitch returns a
    # physically impossible leaf distribution)
    nc1 = _build_routing_nc()
    in1 = _host_prep_routing(x, node_weights, node_biases)
    for attempt in range(2):
        res1 = run_bass_kernel_spmd(nc1, in1, core_ids=list(range(N_CORES)))
        leaves = np.concatenate(
            [res1.results[c]["leaf"] for c in range(N_CORES)]).astype(np.int64)
        if (leaves >= 0).all() and (leaves < N_LEAVES).all():
            counts = np.bincount(leaves, minlength=N_LEAVES)
            if counts.max() <= 64 or attempt == 1:
                break

    # launch 2: expert MLP with capacity from the actual distribution
    spgs = []
    for c in range(N_CORES):
        lo, hi = LEAVES_PER_CORE * c, LEAVES_PER_CORE * (c + 1)
        l_loc = leaves[(leaves >= lo) & (leaves < hi)] - lo
        loads = np.bincount(l_loc, minlength=LEAVES_PER_CORE)
        perm = _lpt_groups(loads)
        gl = loads[perm].reshape(GROUPS, EXPERTS_PER_GROUP).sum(1)
        spgs.append(int(gl.max()))
    need = max(spgs)
    spg = 64 if need <= 64 else int(-(-need // 16) * 16)
    global LAST_SPG
    LAST_SPG = spg
    nc2 = _build_mlp_nc(spg)
    in2, slot_maps = _host_prep_mlp(leaves, x, w1s, b1s, w2s, spg)
    res2 = run_bass_kernel_spmd(nc2, in2, core_ids=list(range(N_CORES)))

    out = np.zeros((BATCH, OUT_W), np.float32)
    for c in range(N_CORES):
        o_slots = np.asarray(res2.results[c]["o"]).astype(np.float32)
        sm = slot_maps[c]
        valid = sm >= 0
        out[sm[valid]] = o_slots[valid]
    return out


# revision 70
# speedup vs baseline: 1.0008x; 1.0008x over previous
"""FFF (fast feedforward / MoE tree-routing) Trainium2 kernel.

Strategy (8 NeuronCores, SPMD, two launches):
  Launch 1 — routing, data-parallel over batch: each core routes 1024 samples
    through the depth-11 plane tree. Levels 0..7 are evaluated densely
    (fp32 matmuls of x against 255 node planes; per-sample select via a single
    fused (iota==cur)*score scalar_tensor_tensor with accum per level).
    Levels 8..10 gather each sample's [w|b] node row with bulk SWDGE
    dma_gathers; the gather index vector is relayouted via a tiny DRAM
    round trip (1 write + 1 wrapped read) and replicated across the 8
    gpsimd core groups with a constant [16,128] PE matmul. Per-sample dots
    are single fused DVE scalar_tensor_tensor+accum ops. Four independent
    quarter-pipelines overlap gather DMA with other quarters' compute.
  Host — slot assignment: samples grouped by leaf expert; leaves sharded
    expert-parallel 256/core; experts are LPT-packed into 32 groups of 8 to
    balance per-group sample counts; capacity spg chosen from the actual
    distribution. x rows gathered+transposed on the host (bf16).
  Launch 2 — expert MLP, expert-parallel, bf16: per 8-expert group one fused
    [768x128] @ [768xspg] bf16 matmul chain computes all 8 experts' h lanes,
    relu+bias on ACT, lane-mask on DVE, then h.T @ W2stack (bf16) produces
    output rows; PSUM drains split ACT/DVE; bf16 stores. Weights stream
    through SBUF once per core (12.6 MB bf16).
  Host — scatter output rows back to sample order (fp32).
"""

import contextlib
import numpy as np
import ml_dtypes

import concourse.bacc as bacc
import concourse.mybir as mybir
import concourse.tile as tile
from concourse.bass import ts
from concourse.mybir import AluOpType
from concourse.bass_utils import run_bass_kernel_spmd

# problem shapes (hardcoded per contract)
DEPTH = 11
IN_W = 768
LEAF_W = 16
OUT_W = 768
N_NODES = 2047
N_LEAVES = 2048
BATCH = 8192
N_CORES = 8

# routing kernel layout
B_CORE = BATCH // N_CORES            # 1024
EXT = 832                            # gather row [w(768) | b | pad] (3328B, %256)
DOT = IN_W + 1                       # useful columns of a gathered row
DENSE_LEVELS = 8                     # levels 0..7 dense (255 nodes)
N_DENSE = 2 ** DENSE_LEVELS - 1      # 255
KC = IN_W // 128                     # 6 contraction k-tiles
NQ = 8                               # routing quarter pipelines
CQ = 8 // NQ                         # c-tiles per quarter (2)
QN = B_CORE // NQ                    # samples per quarter (256)

# mlp kernel layout
LEAVES_PER_CORE = N_LEAVES // N_CORES           # 256
EXPERTS_PER_GROUP = 8
GROUPS = LEAVES_PER_CORE // EXPERTS_PER_GROUP   # 32
SLOTS_PER_GROUP = 64                            # default capacity (exact spg
                                                # picked from the routing result)

F32 = mybir.dt.float32
I32 = mybir.dt.int32
I16 = mybir.dt.int16
BF16 = mybir.dt.bfloat16
BF = ml_dtypes.bfloat16

LAST_SPG = SLOTS_PER_GROUP   # capacity used by the most recent kernel() call


# ---------------------------------------------------------------- launch 1
def _build_routing_nc():
    nc = bacc.Bacc("TRN2", target_bir_lowering=False, debug=False,
                   num_devices=N_CORES)
    xT = nc.dram_tensor("xT", [IN_W, B_CORE], F32, kind="ExternalInput").ap()
    xe = nc.dram_tensor("xe", [B_CORE, DOT], F32, kind="ExternalInput").ap()
    wd = nc.dram_tensor("wd", [IN_W, 256], F32, kind="ExternalInput").ap()
    brep = nc.dram_tensor("brep", [128, 256], F32, kind="ExternalInput").ap()
    RT2 = nc.dram_tensor("RT2", [128, 128], F32, kind="ExternalInput").ap()
    M8 = nc.dram_tensor("M8", [128, CQ * 8], F32, kind="ExternalInput").ap()
    nwe = nc.dram_tensor("nwe", [N_LEAVES, EXT], F32, kind="ExternalInput").ap()
    leaf = nc.dram_tensor("leaf", [B_CORE], I32, kind="ExternalOutput").ap()

    xT_r = xT.rearrange("(k p) s -> p k s", p=128)
    xe_r = xe.rearrange("(c p) d -> p c d", p=128)

    with tile.TileContext(nc) as tc, contextlib.ExitStack() as ctx:
        pool = ctx.enter_context(tc.tile_pool(name="sbuf", bufs=1))
        wpool = ctx.enter_context(tc.tile_pool(name="work", bufs=2))
        cpool = ctx.enter_context(tc.tile_pool(name="cwork", bufs=3))
        psum = ctx.enter_context(tc.tile_pool(name="psum", bufs=1, space="PSUM"))
        psr = ctx.enter_context(tc.tile_pool(name="psr", bufs=2, space="PSUM"))

        # ---- PE warm-up: garbage matmuls ramp the tensor engine to full
        # p-state while the input DMAs stream, so the dense chains that gate
        # the whole kernel run at 1x cycle time from their first instruction.
        psjp = ctx.enter_context(tc.tile_pool(name="psj", bufs=1, space="PSUM"))
        junk = pool.tile([128, 256], F32, name="junk")
        nc.vector.memset(junk[:], 0)
        psj = psjp.tile([128, 128], F32, space="PSUM", name="psjunk")
        for i in range(9):
            nc.tensor.matmul(psj[:], lhsT=junk[:, :128], rhs=junk[:, 128:],
                             start=(i == 0), stop=(i == 8),
                             skip_group_check=True)

        # ---- loads (SP queue, no waits) in consumption order
        wd_sb = pool.tile([128, KC, 256], F32)
        nc.sync.dma_start(out=wd_sb[:], in_=wd.rearrange("(k p) n -> p k n", p=128))
        brep_sb = pool.tile([128, 256], F32)
        nc.sync.dma_start(out=brep_sb[:], in_=brep[:])
        RT2_sb = pool.tile([128, 128], F32)
        nc.sync.dma_start(out=RT2_sb[:], in_=RT2[:])
        M8_sb = pool.tile([128, CQ, 8], F32)
        nc.sync.dma_start(out=M8_sb[:], in_=M8.rearrange("p (c g) -> p c g", g=8))
        xT_q = []
        for q in range(NQ):
            t = pool.tile([128, KC, QN], F32, name=f"xT{q}")
            nc.sync.dma_start(out=t[:], in_=xT_r[:, :, ts(q, QN)])
            xT_q.append(t)
        xe_q = []
        for q in range(NQ):
            t = pool.tile([128, CQ, DOT], F32, name=f"xe{q}")
            nc.sync.dma_start(out=t[:], in_=xe_r[:, ts(q, CQ), :])
            xe_q.append(t)

        # iota of global node index (values = column j)
        iota_i = pool.tile([128, 256], I32)
        nc.gpsimd.iota(iota_i[:], pattern=[[1, 256]], base=0, channel_multiplier=0)
        iota_f = pool.tile([128, 256], F32)
        nc.vector.tensor_copy(out=iota_f[:], in_=iota_i[:])

        # ---- dense matmuls, interleaved with per-quarter trip matmuls so a
        # quarter's level-8 gather issues as soon as its two c-tiles are done
        ps_c = {}

        # split each c-tile's node columns at level boundaries so selects for
        # levels 1..5 start as soon as the first 63-col chain stops
        CH = ((0, 63), (63, 127), (127, 256))

        def dense_mm(c):
            ps = psum.tile([128, 256], F32, space="PSUM", tag=f"ps{c % 3}",
                           name=f"ps{c}")
            for lo, hi in CH:
                for k in range(KC):
                    nc.tensor.matmul(ps[:, lo:hi],
                                     lhsT=xT_q[c // CQ][:, k, ts(c % CQ, 128)],
                                     rhs=wd_sb[:, k, lo:hi], start=(k == 0),
                                     stop=(k == KC - 1))
            ps_c[c] = ps

        # ---- dense selects per quarter (DVE); cur lives in a shared
        # [128, 8] tile (per-quarter column slices) so one DMA per level can
        # export all quarters' indices.
        s_q, sel_q, ch_q = {}, {}, {}
        scr_sel, dot_scr = {}, {}
        cur_l = {0: cpool.tile([128, 8], F32, tag="cur", name="curl0")}
        for q in range(NQ):
            s_q[q] = pool.tile([128, CQ, 256], F32, name=f"s{q}")
            sel_q[q] = pool.tile([128, CQ], F32, name=f"sel{q}")
            ch_q[q] = pool.tile([128, CQ], F32, name=f"ch{q}")
            scr_sel[q] = pool.tile([128, 256], F32, name=f"scrsel{q}")
            dot_scr[q] = pool.tile([128, DOT], F32, name=f"dotscr{q}")
        for lvl in range(1, DENSE_LEVELS):
            cur_l[lvl] = cpool.tile([128, 8], F32, tag="cur", name=f"curl{lvl}")

        def sel_engine(q):
            # odd eighths run their select chains on the (otherwise idle)
            # gpsimd engine so the two sets of chains advance in parallel
            return nc.vector

        def dense_selects(q):
            eng = sel_engine(q)
            for lo, hi in CH:
                for c in range(CQ):
                    # s = psum + bias (per-node row, replicated on partitions)
                    eng.scalar_tensor_tensor(
                        out=s_q[q][:, c, lo:hi], in0=ps_c[CQ * q + c][:, lo:hi],
                        scalar=0.0, in1=brep_sb[:, lo:hi], op0=AluOpType.add,
                        op1=AluOpType.add)
                if lo == 0:
                    # level 0: cur = (s[:,:,0] >= 0) + 1
                    eng.tensor_scalar(
                        out=cur_l[0][:, ts(q, CQ)], in0=s_q[q][:, :, 0],
                        scalar1=0.0, scalar2=1.0, op0=AluOpType.is_ge,
                        op1=AluOpType.add)
                    for lvl in range(1, 6):
                        level_select(q, lvl)
                elif lo == 63:
                    level_select(q, 6)
                else:
                    level_select(q, 7)

        def level_select(q, lvl):
            eng = sel_engine(q)
            if True:
                n = 2 ** lvl
                off = n - 1
                for c in range(CQ):
                    # fused select: sum_j (iota_j == cur) * s_j
                    eng.scalar_tensor_tensor(
                        out=scr_sel[q][:, :n], in0=iota_f[:, off:off + n],
                        scalar=cur_l[lvl - 1][:, CQ * q + c:CQ * q + c + 1],
                        in1=s_q[q][:, c, off:off + n],
                        op0=AluOpType.is_equal, op1=AluOpType.mult,
                        accum_out=sel_q[q][:, c:c + 1])
                eng.tensor_scalar(
                    out=ch_q[q][:], in0=sel_q[q][:], scalar1=0.0, scalar2=1.0,
                    op0=AluOpType.is_ge, op1=AluOpType.add)
                eng.scalar_tensor_tensor(
                    out=cur_l[lvl][:, ts(q, CQ)],
                    in0=cur_l[lvl - 1][:, ts(q, CQ)], scalar=2.0,
                    in1=ch_q[q][:], op0=AluOpType.mult, op1=AluOpType.add)

        # ---- deep levels 8..10: four quarter pipelines, all on-chip.
        # The wrapped [16, 16] gather-index layout is produced by a matmul
        # fold (rhs = cur broadcast * M8 lane mask, F = R'^T @ rhs puts
        # cur[16g+l, c] at [l, 8c+g]), then a second matmul replicates it to
        # the 8 gpsimd core groups. No DMA round trips.
        def q_trip(lvl, q, prev):
            rhs = wpool.tile([128, CQ, 8], F32, tag=f"rhs{q}",
                             name=f"rhs{q}l{lvl}")
            nc.vector.tensor_tensor(
                out=rhs[:], in0=prev[:, ts(q, CQ), None].to_broadcast(
                    [128, CQ, 8]),
                in1=M8_sb[:], op=AluOpType.mult)
            # single fold+replicate matmul: out[m, 8c+g] = cur[16g + m%16, c]
            prep = psr.tile([128, QN // 16], F32, space="PSUM", tag="pr",
                            name=f"pr{q}l{lvl}")
            nc.tensor.matmul(prep[:], lhsT=RT2_sb[:],
                             rhs=rhs.rearrange("p c g -> p (c g)"),
                             start=True, stop=True)
            idx16 = wpool.tile([128, QN // 16], I16, tag=f"ix{q}",
                               name=f"ix{q}l{lvl}")
            nc.scalar.copy(out=idx16[:], in_=prep[:])
            gath = wpool.tile([128, CQ, EXT], F32, tag=f"g{q}",
                              name=f"g{q}l{lvl}")
            nc.gpsimd.dma_gather(
                out_ap=gath[:], in_ap=nwe[:], idxs_ap=idx16[:],
                num_idxs=QN, num_idxs_reg=QN, elem_size=EXT)
            return gath

        def q_dots(lvl, q, prev, nxt, gath):
            for c in range(CQ):
                # fused dot: sum_i xe_i * w_i  (incl. ones-col * bias)
                nc.vector.scalar_tensor_tensor(
                    out=dot_scr[q][:], in0=xe_q[q][:, c, :],
                    scalar=1.0, in1=gath[:, c, :DOT],
                    op0=AluOpType.mult, op1=AluOpType.mult,
                    accum_out=sel_q[q][:, c:c + 1])
            nc.vector.tensor_scalar(
                out=ch_q[q][:], in0=sel_q[q][:], scalar1=0.0, scalar2=1.0,
                op0=AluOpType.is_ge, op1=AluOpType.add)
            nc.vector.scalar_tensor_tensor(
                out=nxt[:, ts(q, CQ)], in0=prev[:, ts(q, CQ)], scalar=2.0,
                in1=ch_q[q][:], op0=AluOpType.mult, op1=AluOpType.add)

        for lvl in range(DENSE_LEVELS, DEPTH):
            cur_l[lvl] = cpool.tile([128, 8], F32, tag="cur", name=f"curl{lvl}")

        gaths = {}
        cur7 = cur_l[DENSE_LEVELS - 1]
        for c in range(CQ):
            dense_mm(c)
        for q in range(NQ):
            if q + 1 < NQ:
                for c in range(CQ * (q + 1), CQ * (q + 2)):
                    dense_mm(c)
            dense_selects(q)
            gaths[q] = q_trip(DENSE_LEVELS, q, cur7)
        for lvl in range(DENSE_LEVELS, DEPTH):
            for q in range(NQ):
                q_dots(lvl, q, cur_l[lvl - 1], cur_l[lvl], gaths[q])
                if lvl + 1 < DEPTH:
                    gaths[q] = q_trip(lvl + 1, q, cur_l[lvl])

        # ---- leaves = cur - N_NODES
        leaf_i = pool.tile([128, 8], I32)
        nc.vector.tensor_scalar(
            out=leaf_i[:], in0=cur_l[DEPTH - 1][:],
            scalar1=float(N_NODES), scalar2=None, op0=AluOpType.subtract)
        nc.sync.dma_start(out=leaf.rearrange("(c p) -> p c", p=128), in_=leaf_i[:])

    nc.compile()
    return nc


def _host_prep_routing(x, node_weights, node_biases):
    wd = np.zeros((IN_W, 256), np.float32)
    wd[:, :N_DENSE] = node_weights[:N_DENSE].T
    brep = np.zeros((128, 256), np.float32)
    brep[:, :N_DENSE] = node_biases[None, :N_DENSE]
    RT2 = (np.arange(128)[:, None] % 16 ==
           np.arange(128)[None, :] % 16).astype(np.float32)
    M8 = np.zeros((128, CQ * 8), np.float32)
    for c in range(CQ):
        M8[np.arange(128), 8 * c + np.arange(128) // 16] = 1.0
    nwe = np.zeros((N_LEAVES, EXT), np.float32)
    nwe[:N_NODES, :IN_W] = node_weights
    nwe[:N_NODES, IN_W] = node_biases

    in_maps = []
    for c in range(N_CORES):
        xs = x[c * B_CORE:(c + 1) * B_CORE]
        xT = np.ascontiguousarray(xs.T)
        xe = np.ones((B_CORE, DOT), np.float32)
        xe[:, :IN_W] = xs
        in_maps.append({"xT": xT, "xe": xe, "wd": wd, "brep": brep,
                        "RT2": RT2, "M8": M8, "nwe": nwe})
    return in_maps


# ---------------------------------------------------------------- launch 2
def _build_mlp_nc(spg=SLOTS_PER_GROUP):
    SLOTS = GROUPS * spg
    NH = OUT_W // 2
    nc = bacc.Bacc("TRN2", target_bir_lowering=False, debug=False,
                   num_devices=N_CORES)
    xgT = nc.dram_tensor("xgT", [IN_W, SLOTS], BF16, kind="ExternalInput").ap()
    wslab = nc.dram_tensor("wslab", [GROUPS, 128, KC * 128 + OUT_W], BF16,
                           kind="ExternalInput").ap()
    b1bc = nc.dram_tensor("b1bc", [128, GROUPS], F32, kind="ExternalInput").ap()
    maskt = nc.dram_tensor("maskt", [128, SLOTS], BF16, kind="ExternalInput").ap()
    out = nc.dram_tensor("o", [SLOTS, OUT_W], BF16, kind="ExternalOutput").ap()

    xgT_r = xgT.rearrange("(k p) s -> p k s", p=128)
    wslab_r = wslab.rearrange("g p m -> p g m")
    NW = 4                      # weight groups per slab DMA
    NSLAB = GROUPS // NW        # 8 slab DMAs
    pair_stores = (spg == 64)   # 2-group packed stores need partition offset 64

    with tile.TileContext(nc) as tc, contextlib.ExitStack() as ctx:
        pool = ctx.enter_context(tc.tile_pool(name="sbuf", bufs=1))
        wpool = ctx.enter_context(tc.tile_pool(name="w", bufs=NSLAB))
        hpool = ctx.enter_context(tc.tile_pool(name="h", bufs=3))
        opool = ctx.enter_context(tc.tile_pool(name="o", bufs=6))
        ps1 = ctx.enter_context(tc.tile_pool(name="ps1", bufs=3, space="PSUM"))
        ps2 = ctx.enter_context(tc.tile_pool(name="ps2", bufs=2, space="PSUM"))

        # loads: interleave xt chunks and weight slabs in consumption order
        xt_sb = []
        mask_sb = pool.tile([128, SLOTS], BF16)
        b1_sb = pool.tile([128, GROUPS], F32)
        w_sb = []
        xt0 = pool.tile([128, KC, NW * spg], BF16, name="xt0")
        nc.sync.dma_start(out=xt0[:], in_=xgT_r[:, :, :NW * spg])
        xt_sb.append(xt0)
        nc.sync.dma_start(out=mask_sb[:], in_=maskt[:])
        nc.sync.dma_start(out=b1_sb[:], in_=b1bc[:])
        for j in range(NSLAB):
            w4 = wpool.tile([128, NW, KC * 128 + OUT_W], BF16, tag="w",
                            name=f"w4_{j}")
            if j == NSLAB - 1:
                for r in range(NW):
                    nc.sync.dma_start(out=w4[:, r, :],
                                      in_=wslab_r[:, j * NW + r, :])
            else:
                nc.sync.dma_start(out=w4[:], in_=wslab_r[:, ts(j, NW), :])
            w_sb.append(w4)
            if j + 1 < NSLAB:
                lo = (j + 1) * NW * spg
                xtj = pool.tile([128, KC, NW * spg], BF16, name=f"xt{j + 1}")
                nc.sync.dma_start(out=xtj[:], in_=xgT_r[:, :, lo:lo + NW * spg])
                xt_sb.append(xtj)

        # pipelined group loop: w1(t) | w2(t-1) | relu+mask(t) | drain(t-2)
        p1_t, hf_t, p2_t, o_t = {}, {}, {}, {}

        def w1mm(g):
            w1 = w_sb[g // NW][:, g % NW, :KC * 128]
            p1 = ps1.tile([128, spg], F32, space="PSUM", tag="p1",
                          name=f"p1_{g}")
            for k in range(KC):
                nc.tensor.matmul(p1[:], lhsT=w1[:, ts(k, 128)],
                                 rhs=xt_sb[g // NW][:, k, ts(g % NW, spg)],
                                 start=(k == 0), stop=(k == KC - 1))
            p1_t[g] = p1

        def hcompute(g):
            h1 = hpool.tile([128, spg], BF16, tag="h1", name=f"h1_{g}")
            nc.scalar.activation(out=h1[:], in_=p1_t[g][:],
                                 func=mybir.ActivationFunctionType.Relu,
                                 bias=b1_sb[:, g:g + 1])
            hf = hpool.tile([128, spg], BF16, tag="hf", name=f"hf_{g}")
            nc.gpsimd.tensor_tensor(out=hf[:], in0=h1[:],
                                    in1=mask_sb[:, ts(g, spg)],
                                    op=AluOpType.mult)
            hf_t[g] = hf

        def w2mm(g):
            w2 = w_sb[g // NW][:, g % NW, KC * 128:]
            p2a = ps2.tile([spg, NH], F32, space="PSUM", tag="p2a",
                           name=f"p2a_{g}")
            p2b = ps2.tile([spg, NH], F32, space="PSUM", tag="p2b",
                           name=f"p2b_{g}")
            nc.tensor.matmul(p2a[:], lhsT=hf_t[g][:], rhs=w2[:, :NH],
                             start=True, stop=True)
            nc.tensor.matmul(p2b[:], lhsT=hf_t[g][:], rhs=w2[:, NH:],
                             start=True, stop=True)
            p2_t[g] = (p2a, p2b)

        def drain(g):
            p2a, p2b = p2_t.pop(g)
            if pair_stores:
                if g % 2 == 0:
                    o_t[g] = opool.tile([128, OUT_W], BF16, tag="o",
                                        name=f"o_{g}")
                o_sb = o_t[g - g % 2]
                off = (g % 2) * spg
                nc.scalar.copy(out=o_sb[off:off + spg, :NH], in_=p2a[:])
                nc.vector.tensor_copy(out=o_sb[off:off + spg, NH:], in_=p2b[:])
                if g % 2 == 1:
                    nc.sync.dma_start(out=out[ts(g // 2, 2 * spg), :],
                                      in_=o_sb[:])
            else:
                o_sb = opool.tile([spg, OUT_W], BF16, tag="o", name=f"o_{g}")
                nc.scalar.copy(out=o_sb[:, :NH], in_=p2a[:])
                nc.vector.tensor_copy(out=o_sb[:, NH:], in_=p2b[:])
                nc.sync.dma_start(out=out[ts(g, spg), :], in_=o_sb[:])

        for t in range(GROUPS + 2):
            if t < GROUPS:
                w1mm(t)
            if 1 <= t <= GROUPS:
                w2mm(t - 1)
            if t < GROUPS:
                hcompute(t)
            if t >= 2:
                drain(t - 2)

    nc.compile()
    return nc


def _host_prep_mlp(leaves, x, w1s, b1s, w2s, spg=SLOTS_PER_GROUP):
    SLOTS = GROUPS * spg
    in_maps, slot_maps = [], []
    order = np.argsort(leaves, kind="stable")
    sorted_leaves = leaves[order]
    for c in range(N_CORES):
        lo, hi = LEAVES_PER_CORE * c, LEAVES_PER_CORE * (c + 1)
        beg, end = np.searchsorted(sorted_leaves, [lo, hi])
        samples = order[beg:end]
        l_loc = leaves[samples] - lo

        # LPT pack experts into groups of 8 balancing sample counts
        loads = np.bincount(l_loc, minlength=LEAVES_PER_CORE)
        perm = _lpt_groups(loads)
        # position of each expert: group + lane-slot within group
        e_group = np.empty(LEAVES_PER_CORE, np.int64)
        e_pos = np.empty(LEAVES_PER_CORE, np.int64)
        for g in range(GROUPS):
            for p_, e in enumerate(perm[g * EXPERTS_PER_GROUP:
                                        (g + 1) * EXPERTS_PER_GROUP]):
                e_group[e] = g
                e_pos[e] = p_

        g_all = e_group[l_loc]
        p_all = e_pos[l_loc]
        slot = np.empty(len(samples), np.int64)
        fill = np.zeros(GROUPS, np.int64)
        for i, g in enumerate(g_all):
            slot[i] = spg * g + fill[g]
            fill[g] += 1
        assert not len(fill) or fill.max() <= spg

        slot_sample = np.full(SLOTS, -1, np.int64)
        slot_sample[slot] = samples
        mask = np.zeros((128, SLOTS), BF)
        lane_rows = (16 * p_all[None, :] + np.arange(16)[:, None])
        mask[lane_rows, slot[None, :]] = 1.0

        xg = np.zeros((SLOTS, IN_W), np.float32)
        xg[slot] = x[samples]
        xgT = np.ascontiguousarray(xg.T).astype(BF)

        eids = lo + perm                       # global leaf id per lane-slot
        w1sel = w1s[eids]                      # [256, 768, 16]
        w1f = (w1sel.reshape(GROUPS, 8, IN_W, LEAF_W)
               .transpose(0, 2, 1, 3)
               .reshape(GROUPS, IN_W, 128)
               .reshape(GROUPS, KC, 128, 128)
               .transpose(0, 2, 1, 3)
               .reshape(GROUPS, 128, KC * 128))
        w2f = w2s[eids].reshape(GROUPS, 128, OUT_W)
        wslab = np.ascontiguousarray(
            np.concatenate([w1f, w2f], axis=2)).astype(BF)
        b1bc = np.ascontiguousarray(
            b1s[eids].reshape(GROUPS, 128).T).astype(np.float32)

        in_maps.append({"xgT": xgT, "wslab": wslab, "b1bc": b1bc,
                        "maskt": mask})
        slot_maps.append(slot_sample)
    return in_maps, slot_maps


def _lpt_groups(loads):
    """Pack 256 experts into 32 groups of exactly 8, balancing total load
    (greedy LPT with cardinality cap). Returns expert permutation, 8 per
    group."""
    order = np.argsort(-loads, kind="stable")
    gload = np.zeros(GROUPS, np.int64)
    gcnt = np.zeros(GROUPS, np.int64)
    members = [[] for _ in range(GROUPS)]
    for e in order:
        gl = np.where(gcnt < EXPERTS_PER_GROUP, gload, np.iinfo(np.int64).max)
        g = int(np.argmin(gl))
        members[g].append(int(e))
        gload[g] += loads[e]
        gcnt[g] += 1
    return np.array([e for g in range(GROUPS) for e in members[g]],
                    dtype=np.int64)


# ---------------------------------------------------------------- entry
def kernel(x, node_weights, node_biases, w1s, b1s, w2s):
    x = np.ascontiguousarray(np.asarray(x, np.float32))
    node_weights = np.ascontiguousarray(np.asarray(node_weights, np.float32))
    node_biases = np.ascontiguousarray(np.asarray(node_biases, np.float32))
    w1s = np.asarray(w1s, np.float32)
    b1s = np.asarray(b1s, np.float32)
    w2s = np.asarray(w2s, np.float32)

    # launch 1: routing (retry once if a transient device glitch returns a
    # physically impossible leaf distribution)
    nc1 = _build_routing_nc()
    in1 = _host_prep_routing(x, node_weights, node_biases)
    for attempt in range(2):
        res1 = run_bass_kernel_spmd(nc1, in1, core_ids=list(range(N_CORES)))
        leaves = np.concatenate(
            [res1.results[c]["leaf"] for c in range(N_CORES)]).astype(np.int64)
        if (leaves >= 0).all() and (leaves < N_LEAVES).all():
            counts = np.bincount(leaves, minlength=N_LEAVES)
            if counts.max() <= 64 or attempt == 1:
                break

    # launch 2: expert MLP with capacity from the actual distribution
    spgs = []
    for c in range(N_CORES):
        lo, hi = LEAVES_PER_CORE * c, LEAVES_PER_CORE * (c + 1)
        l_loc = leaves[(leaves >= lo) & (leaves < hi)] - lo
        loads = np.bincount(l_loc, minlength=LEAVES_PER_CORE)
        perm = _lpt_groups(loads)
        gl = loads[perm].reshape(GROUPS, EXPERTS_PER_GROUP).sum(1)
        spgs.append(int(gl.max()))
    need = max(spgs)
    spg = 64 if need <= 64 else int(-(-need // 16) * 16)
    global LAST_SPG
    LAST_SPG = spg
    nc2 = _build_mlp_nc(spg)
    in2, slot_maps = _host_prep_mlp(leaves, x, w1s, b1s, w2s, spg)
    res2 = run_bass_kernel_spmd(nc2, in2, core_ids=list(range(N_CORES)))

    out = np.zeros((BATCH, OUT_W), np.float32)
    for c in range(N_CORES):
        o_slots = np.asarray(res2.results[c]["o"]).astype(np.float32)
        sm = slot_maps[c]
        valid = sm >= 0
        out[sm[valid]] = o_slots[valid]
    return out


# revision 75
# speedup vs baseline: 1.0030x; 1.0022x over previous
"""FFF (fast feedforward / MoE tree-routing) Trainium2 kernel.

Strategy (8 NeuronCores, SPMD, two launches):
  Launch 1 — routing, data-parallel over batch: each core routes 1024 samples
    through the depth-11 plane tree. Levels 0..7 are evaluated densely
    (fp32 matmuls of x against 255 node planes; per-sample select via a single
    fused (iota==cur)*score scalar_tensor_tensor with accum per level).
    Levels 8..10 gather each sample's [w|b] node row with bulk SWDGE
    dma_gathers; the gather index vector is relayouted via a tiny DRAM
    round trip (1 write + 1 wrapped read) and replicated across the 8
    gpsimd core groups with a constant [16,128] PE matmul. Per-sample dots
    are single fused DVE scalar_tensor_tensor+accum ops. Four independent
    quarter-pipelines overlap gather DMA with other quarters' compute.
  Host — slot assignment: samples grouped by leaf expert; leaves sharded
    expert-parallel 256/core; experts are LPT-packed into 32 groups of 8 to
    balance per-group sample counts; capacity spg chosen from the actual
    distribution. x rows gathered+transposed on the host (bf16).
  Launch 2 — expert MLP, expert-parallel, bf16: per 8-expert group one fused
    [768x128] @ [768xspg] bf16 matmul chain computes all 8 experts' h lanes,
    relu+bias on ACT, lane-mask on DVE, then h.T @ W2stack (bf16) produces
    output rows; PSUM drains split ACT/DVE; bf16 stores. Weights stream
    through SBUF once per core (12.6 MB bf16).
  Host — scatter output rows back to sample order (fp32).
"""

import contextlib
import numpy as np
import ml_dtypes

import concourse.bacc as bacc
import concourse.mybir as mybir
import concourse.tile as tile
from concourse.bass import ts
from concourse.mybir import AluOpType
from concourse.bass_utils import run_bass_kernel_spmd

# problem shapes (hardcoded per contract)
DEPTH = 11
IN_W = 768
LEAF_W = 16
OUT_W = 768
N_NODES = 2047
N_LEAVES = 2048
BATCH = 8192
N_CORES = 8

# routing kernel layout
B_CORE = BATCH // N_CORES            # 1024
EXT = 832                            # gather row [w(768) | b | pad] (3328B, %256)
DOT = IN_W + 1                       # useful columns of a gathered row
DENSE_LEVELS = 8                     # levels 0..7 dense (255 nodes)
N_DENSE = 2 ** DENSE_LEVELS - 1      # 255
KC = IN_W // 128                     # 6 contraction k-tiles
NQ = 8                               # routing quarter pipelines
CQ = 8 // NQ                         # c-tiles per quarter (2)
QN = B_CORE // NQ                    # samples per quarter (256)

# mlp kernel layout
LEAVES_PER_CORE = N_LEAVES // N_CORES           # 256
EXPERTS_PER_GROUP = 8
GROUPS = LEAVES_PER_CORE // EXPERTS_PER_GROUP   # 32
SLOTS_PER_GROUP = 64                            # default capacity (exact spg
                                                # picked from the routing result)

F32 = mybir.dt.float32
I32 = mybir.dt.int32
I16 = mybir.dt.int16
BF16 = mybir.dt.bfloat16
BF = ml_dtypes.bfloat16

LAST_SPG = SLOTS_PER_GROUP   # capacity used by the most recent kernel() call


# ---------------------------------------------------------------- launch 1
def _build_routing_nc():
    nc = bacc.Bacc("TRN2", target_bir_lowering=False, debug=False,
                   num_devices=N_CORES)
    xT = nc.dram_tensor("xT", [IN_W, B_CORE], F32, kind="ExternalInput").ap()
    xe = nc.dram_tensor("xe", [B_CORE, DOT], F32, kind="ExternalInput").ap()
    wd = nc.dram_tensor("wd", [IN_W, 256], F32, kind="ExternalInput").ap()
    brep = nc.dram_tensor("brep", [128, 256], F32, kind="ExternalInput").ap()
    RT2 = nc.dram_tensor("RT2", [128, 128], F32, kind="ExternalInput").ap()
    M8 = nc.dram_tensor("M8", [128, CQ * 8], F32, kind="ExternalInput").ap()
    nwe = nc.dram_tensor("nwe", [N_LEAVES, EXT], F32, kind="ExternalInput").ap()
    leaf = nc.dram_tensor("leaf", [B_CORE], I32, kind="ExternalOutput").ap()

    xT_r = xT.rearrange("(k p) s -> p k s", p=128)
    xe_r = xe.rearrange("(c p) d -> p c d", p=128)

    with tile.TileContext(nc) as tc, contextlib.ExitStack() as ctx:
        pool = ctx.enter_context(tc.tile_pool(name="sbuf", bufs=1))
        wpool = ctx.enter_context(tc.tile_pool(name="work", bufs=2))
        cpool = ctx.enter_context(tc.tile_pool(name="cwork", bufs=3))
        psum = ctx.enter_context(tc.tile_pool(name="psum", bufs=1, space="PSUM"))
        psr = ctx.enter_context(tc.tile_pool(name="psr", bufs=2, space="PSUM"))

        # ---- PE warm-up: garbage matmuls ramp the tensor engine to full
        # p-state while the input DMAs stream, so the dense chains that gate
        # the whole kernel run at 1x cycle time from their first instruction.
        psjp = ctx.enter_context(tc.tile_pool(name="psj", bufs=1, space="PSUM"))
        junk = pool.tile([128, 256], F32, name="junk")
        nc.vector.memset(junk[:], 0)
        psj = psjp.tile([128, 128], F32, space="PSUM", name="psjunk")
        for i in range(9):
            nc.tensor.matmul(psj[:], lhsT=junk[:, :128], rhs=junk[:, 128:],
                             start=(i == 0), stop=(i == 8),
                             skip_group_check=True)

        # ---- loads (SP queue, no waits) in consumption order
        wd_sb = pool.tile([128, KC, 256], F32)
        nc.sync.dma_start(out=wd_sb[:], in_=wd.rearrange("(k p) n -> p k n", p=128))
        brep_sb = pool.tile([128, 256], F32)
        nc.sync.dma_start(out=brep_sb[:], in_=brep[:])
        RT2_sb = pool.tile([128, 128], F32)
        nc.sync.dma_start(out=RT2_sb[:], in_=RT2[:])
        M8_sb = pool.tile([128, CQ, 8], F32)
        nc.sync.dma_start(out=M8_sb[:], in_=M8.rearrange("p (c g) -> p c g", g=8))
        xT_q = []
        for q in range(NQ):
            t = pool.tile([128, KC, QN], F32, name=f"xT{q}")
            nc.sync.dma_start(out=t[:], in_=xT_r[:, :, ts(q, QN)])
            xT_q.append(t)
        xe_q = []
        for q in range(NQ):
            t = pool.tile([128, CQ, DOT], F32, name=f"xe{q}")
            nc.sync.dma_start(out=t[:], in_=xe_r[:, ts(q, CQ), :])
            xe_q.append(t)

        # iota of global node index (values = column j)
        iota_i = pool.tile([128, 256], I32)
        nc.gpsimd.iota(iota_i[:], pattern=[[1, 256]], base=0, channel_multiplier=0)
        iota_f = pool.tile([128, 256], F32)
        nc.vector.tensor_copy(out=iota_f[:], in_=iota_i[:])

        # ---- dense matmuls, interleaved with per-quarter trip matmuls so a
        # quarter's level-8 gather issues as soon as its two c-tiles are done
        ps_c = {}

        # split each c-tile's node columns at level boundaries so selects for
        # levels 1..5 start as soon as the first 63-col chain stops
        CH = ((0, 256),)

        def dense_mm(c):
            ps = psum.tile([128, 256], F32, space="PSUM", tag=f"ps{c % 3}",
                           name=f"ps{c}")
            for lo, hi in CH:
                for k in range(KC):
                    nc.tensor.matmul(ps[:, lo:hi],
                                     lhsT=xT_q[c // CQ][:, k, ts(c % CQ, 128)],
                                     rhs=wd_sb[:, k, lo:hi], start=(k == 0),
                                     stop=(k == KC - 1))
            ps_c[c] = ps

        # ---- dense selects per quarter (DVE); cur lives in a shared
        # [128, 8] tile (per-quarter column slices) so one DMA per level can
        # export all quarters' indices.
        s_q, sel_q, ch_q = {}, {}, {}
        scr_sel, dot_scr = {}, {}
        cur_l = {0: cpool.tile([128, 8], F32, tag="cur", name="curl0")}
        for q in range(NQ):
            s_q[q] = pool.tile([128, CQ, 256], F32, name=f"s{q}")
            sel_q[q] = pool.tile([128, CQ], F32, name=f"sel{q}")
            ch_q[q] = pool.tile([128, CQ], F32, name=f"ch{q}")
            scr_sel[q] = pool.tile([128, 256], F32, name=f"scrsel{q}")
            dot_scr[q] = pool.tile([128, DOT], F32, name=f"dotscr{q}")
        for lvl in range(1, DENSE_LEVELS):
            cur_l[lvl] = cpool.tile([128, 8], F32, tag="cur", name=f"curl{lvl}")

        def sel_engine(q):
            # odd eighths run their select chains on the (otherwise idle)
            # gpsimd engine so the two sets of chains advance in parallel
            return nc.vector

        def dense_selects(q):
            eng = sel_engine(q)
            for lo, hi in CH:
                for c in range(CQ):
                    # s = psum + bias (per-node row, replicated on partitions)
                    eng.scalar_tensor_tensor(
                        out=s_q[q][:, c, lo:hi], in0=ps_c[CQ * q + c][:, lo:hi],
                        scalar=0.0, in1=brep_sb[:, lo:hi], op0=AluOpType.add,
                        op1=AluOpType.add)
                if lo == 0:
                    # level 0: cur = (s[:,:,0] >= 0) + 1
                    eng.tensor_scalar(
                        out=cur_l[0][:, ts(q, CQ)], in0=s_q[q][:, :, 0],
                        scalar1=0.0, scalar2=1.0, op0=AluOpType.is_ge,
                        op1=AluOpType.add)
                # run every level whose node columns are fully copied
                for lvl in range(1, DENSE_LEVELS):
                    if 2 ** lvl - 1 >= lo and 2 ** (lvl + 1) - 1 <= hi:
                        level_select(q, lvl)

        def level_select(q, lvl):
            eng = sel_engine(q)
            if True:
                n = 2 ** lvl
                off = n - 1
                for c in range(CQ):
                    # fused select: sum_j (iota_j == cur) * s_j
                    eng.scalar_tensor_tensor(
                        out=scr_sel[q][:, :n], in0=iota_f[:, off:off + n],
                        scalar=cur_l[lvl - 1][:, CQ * q + c:CQ * q + c + 1],
                        in1=s_q[q][:, c, off:off + n],
                        op0=AluOpType.is_equal, op1=AluOpType.mult,
                        accum_out=sel_q[q][:, c:c + 1])
                eng.tensor_scalar(
                    out=ch_q[q][:], in0=sel_q[q][:], scalar1=0.0, scalar2=1.0,
                    op0=AluOpType.is_ge, op1=AluOpType.add)
                eng.scalar_tensor_tensor(
                    out=cur_l[lvl][:, ts(q, CQ)],
                    in0=cur_l[lvl - 1][:, ts(q, CQ)], scalar=2.0,
                    in1=ch_q[q][:], op0=AluOpType.mult, op1=AluOpType.add)

        # ---- deep levels 8..10: four quarter pipelines, all on-chip.
        # The wrapped [16, 16] gather-index layout is produced by a matmul
        # fold (rhs = cur broadcast * M8 lane mask, F = R'^T @ rhs puts
        # cur[16g+l, c] at [l, 8c+g]), then a second matmul replicates it to
        # the 8 gpsimd core groups. No DMA round trips.
        def q_trip(lvl, q, prev):
            rhs = wpool.tile([128, CQ, 8], F32, tag=f"rhs{q}",
                             name=f"rhs{q}l{lvl}")
            nc.vector.tensor_tensor(
                out=rhs[:], in0=prev[:, ts(q, CQ), None].to_broadcast(
                    [128, CQ, 8]),
                in1=M8_sb[:], op=AluOpType.mult)
            # single fold+replicate matmul: out[m, 8c+g] = cur[16g + m%16, c]
            prep = psr.tile([128, QN // 16], F32, space="PSUM", tag="pr",
                            name=f"pr{q}l{lvl}")
            nc.tensor.matmul(prep[:], lhsT=RT2_sb[:],
                             rhs=rhs.rearrange("p c g -> p (c g)"),
                             start=True, stop=True)
            idx16 = wpool.tile([128, QN // 16], I16, tag=f"ix{q}",
                               name=f"ix{q}l{lvl}")
            nc.scalar.copy(out=idx16[:], in_=prep[:])
            gath = wpool.tile([128, CQ, EXT], F32, tag=f"g{q}",
                              name=f"g{q}l{lvl}")
            nc.gpsimd.dma_gather(
                out_ap=gath[:], in_ap=nwe[:], idxs_ap=idx16[:],
                num_idxs=QN, num_idxs_reg=QN, elem_size=EXT)
            return gath

        def q_dots(lvl, q, prev, nxt, gath):
            for c in range(CQ):
                # fused dot: sum_i xe_i * w_i  (incl. ones-col * bias)
                nc.vector.scalar_tensor_tensor(
                    out=dot_scr[q][:], in0=xe_q[q][:, c, :],
                    scalar=1.0, in1=gath[:, c, :DOT],
                    op0=AluOpType.mult, op1=AluOpType.mult,
                    accum_out=sel_q[q][:, c:c + 1])
            nc.vector.tensor_scalar(
                out=ch_q[q][:], in0=sel_q[q][:], scalar1=0.0, scalar2=1.0,
                op0=AluOpType.is_ge, op1=AluOpType.add)
            nc.vector.scalar_tensor_tensor(
                out=nxt[:, ts(q, CQ)], in0=prev[:, ts(q, CQ)], scalar=2.0,
                in1=ch_q[q][:], op0=AluOpType.mult, op1=AluOpType.add)

        for lvl in range(DENSE_LEVELS, DEPTH):
            cur_l[lvl] = cpool.tile([128, 8], F32, tag="cur", name=f"curl{lvl}")

        gaths = {}
        cur7 = cur_l[DENSE_LEVELS - 1]
        for c in range(CQ):
            dense_mm(c)
        for q in range(NQ):
            if q + 1 < NQ:
                for c in range(CQ * (q + 1), CQ * (q + 2)):
                    dense_mm(c)
            dense_selects(q)
            gaths[q] = q_trip(DENSE_LEVELS, q, cur7)
        for lvl in range(DENSE_LEVELS, DEPTH):
            for q in range(NQ):
                q_dots(lvl, q, cur_l[lvl - 1], cur_l[lvl], gaths[q])
                if lvl + 1 < DEPTH:
                    gaths[q] = q_trip(lvl + 1, q, cur_l[lvl])

        # ---- leaves = cur - N_NODES
        leaf_i = pool.tile([128, 8], I32)
        nc.vector.tensor_scalar(
            out=leaf_i[:], in0=cur_l[DEPTH - 1][:],
            scalar1=float(N_NODES), scalar2=None, op0=AluOpType.subtract)
        nc.sync.dma_start(out=leaf.rearrange("(c p) -> p c", p=128), in_=leaf_i[:])

    nc.compile()
    return nc


def _host_prep_routing(x, node_weights, node_biases):
    wd = np.zeros((IN_W, 256), np.float32)
    wd[:, :N_DENSE] = node_weights[:N_DENSE].T
    brep = np.zeros((128, 256), np.float32)
    brep[:, :N_DENSE] = node_biases[None, :N_DENSE]
    RT2 = (np.arange(128)[:, None] % 16 ==
           np.arange(128)[None, :] % 16).astype(np.float32)
    M8 = np.zeros((128, CQ * 8), np.float32)
    for c in range(CQ):
        M8[np.arange(128), 8 * c + np.arange(128) // 16] = 1.0
    nwe = np.zeros((N_LEAVES, EXT), np.float32)
    nwe[:N_NODES, :IN_W] = node_weights
    nwe[:N_NODES, IN_W] = node_biases

    in_maps = []
    for c in range(N_CORES):
        xs = x[c * B_CORE:(c + 1) * B_CORE]
        xT = np.ascontiguousarray(xs.T)
        xe = np.ones((B_CORE, DOT), np.float32)
        xe[:, :IN_W] = xs
        in_maps.append({"xT": xT, "xe": xe, "wd": wd, "brep": brep,
                        "RT2": RT2, "M8": M8, "nwe": nwe})
    return in_maps


# ---------------------------------------------------------------- launch 2
def _build_mlp_nc(spg=SLOTS_PER_GROUP):
    SLOTS = GROUPS * spg
    NH = OUT_W // 2
    nc = bacc.Bacc("TRN2", target_bir_lowering=False, debug=False,
                   num_devices=N_CORES)
    xgT = nc.dram_tensor("xgT", [IN_W, SLOTS], BF16, kind="ExternalInput").ap()
    wslab = nc.dram_tensor("wslab", [GROUPS, 128, KC * 128 + OUT_W], BF16,
                           kind="ExternalInput").ap()
    b1bc = nc.dram_tensor("b1bc", [128, GROUPS], F32, kind="ExternalInput").ap()
    maskt = nc.dram_tensor("maskt", [128, SLOTS], BF16, kind="ExternalInput").ap()
    out = nc.dram_tensor("o", [SLOTS, OUT_W], BF16, kind="ExternalOutput").ap()

    xgT_r = xgT.rearrange("(k p) s -> p k s", p=128)
    wslab_r = wslab.rearrange("g p m -> p g m")
    NW = 4                      # weight groups per slab DMA
    NSLAB = GROUPS // NW        # 8 slab DMAs
    pair_stores = (spg == 64)   # 2-group packed stores need partition offset 64

    with tile.TileContext(nc) as tc, contextlib.ExitStack() as ctx:
        pool = ctx.enter_context(tc.tile_pool(name="sbuf", bufs=1))
        wpool = ctx.enter_context(tc.tile_pool(name="w", bufs=NSLAB))
        hpool = ctx.enter_context(tc.tile_pool(name="h", bufs=3))
        opool = ctx.enter_context(tc.tile_pool(name="o", bufs=6))
        ps1 = ctx.enter_context(tc.tile_pool(name="ps1", bufs=3, space="PSUM"))
        ps2 = ctx.enter_context(tc.tile_pool(name="ps2", bufs=2, space="PSUM"))

        # loads: interleave xt chunks and weight slabs in consumption order
        xt_sb = []
        mask_sb = pool.tile([128, SLOTS], BF16)
        b1_sb = pool.tile([128, GROUPS], F32)
        w_sb = []
        xt0 = pool.tile([128, KC, NW * spg], BF16, name="xt0")
        nc.sync.dma_start(out=xt0[:], in_=xgT_r[:, :, :NW * spg])
        xt_sb.append(xt0)
        nc.sync.dma_start(out=mask_sb[:], in_=maskt[:])
        nc.sync.dma_start(out=b1_sb[:], in_=b1bc[:])
        for j in range(NSLAB):
            w4 = wpool.tile([128, NW, KC * 128 + OUT_W], BF16, tag="w",
                            name=f"w4_{j}")
            if j == NSLAB - 1:
                for r in range(NW):
                    nc.sync.dma_start(out=w4[:, r, :],
                                      in_=wslab_r[:, j * NW + r, :])
            else:
                nc.sync.dma_start(out=w4[:], in_=wslab_r[:, ts(j, NW), :])
            w_sb.append(w4)
            if j + 1 < NSLAB:
                lo = (j + 1) * NW * spg
                xtj = pool.tile([128, KC, NW * spg], BF16, name=f"xt{j + 1}")
                nc.sync.dma_start(out=xtj[:], in_=xgT_r[:, :, lo:lo + NW * spg])
                xt_sb.append(xtj)

        # pipelined group loop: w1(t) | w2(t-1) | relu+mask(t) | drain(t-2)
        p1_t, hf_t, p2_t, o_t = {}, {}, {}, {}

        def w1mm(g):
            w1 = w_sb[g // NW][:, g % NW, :KC * 128]
            p1 = ps1.tile([128, spg], F32, space="PSUM", tag="p1",
                          name=f"p1_{g}")
            for k in range(KC):
                nc.tensor.matmul(p1[:], lhsT=w1[:, ts(k, 128)],
                                 rhs=xt_sb[g // NW][:, k, ts(g % NW, spg)],
                                 start=(k == 0), stop=(k == KC - 1))
            p1_t[g] = p1

        def hcompute(g):
            h1 = hpool.tile([128, spg], BF16, tag="h1", name=f"h1_{g}")
            nc.scalar.activation(out=h1[:], in_=p1_t[g][:],
                                 func=mybir.ActivationFunctionType.Relu,
                                 bias=b1_sb[:, g:g + 1])
            hf = hpool.tile([128, spg], BF16, tag="hf", name=f"hf_{g}")
            nc.gpsimd.tensor_tensor(out=hf[:], in0=h1[:],
                                    in1=mask_sb[:, ts(g, spg)],
                                    op=AluOpType.mult)
            hf_t[g] = hf

        def w2mm(g):
            w2 = w_sb[g // NW][:, g % NW, KC * 128:]
            p2a = ps2.tile([spg, NH], F32, space="PSUM", tag="p2a",
                           name=f"p2a_{g}")
            p2b = ps2.tile([spg, NH], F32, space="PSUM", tag="p2b",
                           name=f"p2b_{g}")
            nc.tensor.matmul(p2a[:], lhsT=hf_t[g][:], rhs=w2[:, :NH],
                             start=True, stop=True)
            nc.tensor.matmul(p2b[:], lhsT=hf_t[g][:], rhs=w2[:, NH:],
                             start=True, stop=True)
            p2_t[g] = (p2a, p2b)

        def drain(g):
            p2a, p2b = p2_t.pop(g)
            if pair_stores:
                if g % 2 == 0:
                    o_t[g] = opool.tile([128, OUT_W], BF16, tag="o",
                                        name=f"o_{g}")
                o_sb = o_t[g - g % 2]
                off = (g % 2) * spg
                nc.scalar.copy(out=o_sb[off:off + spg, :NH], in_=p2a[:])
                nc.vector.tensor_copy(out=o_sb[off:off + spg, NH:], in_=p2b[:])
                if g % 2 == 1:
                    nc.sync.dma_start(out=out[ts(g // 2, 2 * spg), :],
                                      in_=o_sb[:])
            else:
                o_sb = opool.tile([spg, OUT_W], BF16, tag="o", name=f"o_{g}")
                nc.scalar.copy(out=o_sb[:, :NH], in_=p2a[:])
                nc.vector.tensor_copy(out=o_sb[:, NH:], in_=p2b[:])
                nc.sync.dma_start(out=out[ts(g, spg), :], in_=o_sb[:])

        for t in range(GROUPS + 2):
            if t < GROUPS:
                w1mm(t)
            if 1 <= t <= GROUPS:
                w2mm(t - 1)
            if t < GROUPS:
                hcompute(t)
            if t >= 2:
                drain(t - 2)

    nc.compile()
    return nc


def _host_prep_mlp(leaves, x, w1s, b1s, w2s, spg=SLOTS_PER_GROUP):
    SLOTS = GROUPS * spg
    in_maps, slot_maps = [], []
    order = np.argsort(leaves, kind="stable")
    sorted_leaves = leaves[order]
    for c in range(N_CORES):
        lo, hi = LEAVES_PER_CORE * c, LEAVES_PER_CORE * (c + 1)
        beg, end = np.searchsorted(sorted_leaves, [lo, hi])
        samples = order[beg:end]
        l_loc = leaves[samples] - lo

        # LPT pack experts into groups of 8 balancing sample counts
        loads = np.bincount(l_loc, minlength=LEAVES_PER_CORE)
        perm = _lpt_groups(loads)
        # position of each expert: group + lane-slot within group
        e_group = np.empty(LEAVES_PER_CORE, np.int64)
        e_pos = np.empty(LEAVES_PER_CORE, np.int64)
        for g in range(GROUPS):
            for p_, e in enumerate(perm[g * EXPERTS_PER_GROUP:
                                        (g + 1) * EXPERTS_PER_GROUP]):
                e_group[e] = g
                e_pos[e] = p_

        g_all = e_group[l_loc]
        p_all = e_pos[l_loc]
        slot = np.empty(len(samples), np.int64)
        fill = np.zeros(GROUPS, np.int64)
        for i, g in enumerate(g_all):
            slot[i] = spg * g + fill[g]
            fill[g] += 1
        assert not len(fill) or fill.max() <= spg

        slot_sample = np.full(SLOTS, -1, np.int64)
        slot_sample[slot] = samples
        mask = np.zeros((128, SLOTS), BF)
        lane_rows = (16 * p_all[None, :] + np.arange(16)[:, None])
        mask[lane_rows, slot[None, :]] = 1.0

        xg = np.zeros((SLOTS, IN_W), np.float32)
        xg[slot] = x[samples]
        xgT = np.ascontiguousarray(xg.T).astype(BF)

        eids = lo + perm                       # global leaf id per lane-slot
        w1sel = w1s[eids]                      # [256, 768, 16]
        w1f = (w1sel.reshape(GROUPS, 8, IN_W, LEAF_W)
               .transpose(0, 2, 1, 3)
               .reshape(GROUPS, IN_W, 128)
               .reshape(GROUPS, KC, 128, 128)
               .transpose(0, 2, 1, 3)
               .reshape(GROUPS, 128, KC * 128))
        w2f = w2s[eids].reshape(GROUPS, 128, OUT_W)
        wslab = np.ascontiguousarray(
            np.concatenate([w1f, w2f], axis=2)).astype(BF)
        b1bc = np.ascontiguousarray(
            b1s[eids].reshape(GROUPS, 128).T).astype(np.float32)

        in_maps.append({"xgT": xgT, "wslab": wslab, "b1bc": b1bc,
                        "maskt": mask})
        slot_maps.append(slot_sample)
    return in_maps, slot_maps


def _lpt_groups(loads):
    """Pack 256 experts into 32 groups of exactly 8, balancing total load
    (greedy LPT with cardinality cap). Returns expert permutation, 8 per
    group."""
    order = np.argsort(-loads, kind="stable")
    gload = np.zeros(GROUPS, np.int64)
    gcnt = np.zeros(GROUPS, np.int64)
    members = [[] for _ in range(GROUPS)]
    for e in order:
        gl = np.where(gcnt < EXPERTS_PER_GROUP, gload, np.iinfo(np.int64).max)
        g = int(np.argmin(gl))
        members[g].append(int(e))
        gload[g] += loads[e]
        gcnt[g] += 1
    return np.array([e for g in range(GROUPS) for e in members[g]],
                    dtype=np.int64)


# ---------------------------------------------------------------- entry
def kernel(x, node_weights, node_biases, w1s, b1s, w2s):
    x = np.ascontiguousarray(np.asarray(x, np.float32))
    node_weights = np.ascontiguousarray(np.asarray(node_weights, np.float32))
    node_biases = np.ascontiguousarray(np.asarray(node_biases, np.float32))
    w1s = np.asarray(w1s, np.float32)
    b1s = np.asarray(b1s, np.float32)
    w2s = np.asarray(w2s, np.float32)

    # launch 1: routing (retry once if a transient device glitch returns a
    # physically impossible leaf distribution)
    nc1 = _build_routing_nc()
    in1 = _host_prep_routing(x, node_weights, node_biases)
    for attempt in range(2):
        res1 = run_bass_kernel_spmd(nc1, in1, core_ids=list(range(N_CORES)))
        leaves = np.concatenate(
            [res1.results[c]["leaf"] for c in range(N_CORES)]).astype(np.int64)
        if (leaves >= 0).all() and (leaves < N_LEAVES).all():
            counts = np.bincount(leaves, minlength=N_LEAVES)
            if counts.max() <= 64 or attempt == 1:
                break

    # launch 2: expert MLP with capacity from the actual distribution
    spgs = []
    for c in range(N_CORES):
        lo, hi = LEAVES_PER_CORE * c, LEAVES_PER_CORE * (c + 1)
        l_loc = leaves[(leaves >= lo) & (leaves < hi)] - lo
        loads = np.bincount(l_loc, minlength=LEAVES_PER_CORE)
        perm = _lpt_groups(loads)
        gl = loads[perm].reshape(GROUPS, EXPERTS_PER_GROUP).sum(1)
        spgs.append(int(gl.max()))
    need = max(spgs)
    spg = 64 if need <= 64 else int(-(-need // 16) * 16)
    global LAST_SPG
    LAST_SPG = spg
    nc2 = _build_mlp_nc(spg)
    in2, slot_maps = _host_prep_mlp(leaves, x, w1s, b1s, w2s, spg)
    res2 = run_bass_kernel_spmd(nc2, in2, core_ids=list(range(N_CORES)))

    out = np.zeros((BATCH, OUT_W), np.float32)
    for c in range(N_CORES):
        o_slots = np.asarray(res2.results[c]["o"]).astype(np.float32)
        sm = slot_maps[c]
        valid = sm >= 0
        out[sm[valid]] = o_slots[valid]
    return out
